# revision 15
# baseline (speedup 1.0000x reference)
"""3-layer GCN (100k nodes, 1.6M edges, 128->128->128->40) on 8 trn2 cores.

Self-contained harness kernel: kernel(**inputs) takes the FULL unsharded
inputs and returns the FULL [100000, 40] float32 output.

Strategy (1D node partition, edges sharded by dst):
  - nodes split contiguously across the 8 cores (12500 each, padded 12544 =
    98 windows of 128); edges assigned to the core owning their dst.  Within
    a core, nodes are greedily re-packed into windows so that per-(window,
    bucket) edge counts are balanced across cores -- the SPMD program pads
    every (w,b) run to the max over cores, so balancing shrinks the padding.
  - the gathered table holds RAW scaled node rows tab_l[i] = bf16(h_l[i] *
    ns[i] [* nd[i] for l>0]), 128 bf16 = 256B per row; the weight matmul is
    applied AFTER aggregation (W commutes with segment_sum), so every layer
    gathers 256B rows and the per-node transform pass disappears.
  - tab_0 is computed host-side and fed as a replicated ExternalInput, so
    there is no AllGather before layer 0.  Layers 1/2 AllGather in TWO
    chunks (windows 0-47 / 48-97): the first fires mid-layer (with slack so
    the gpsimd stream never stalls on epilogue stores) and hides under the
    remaining aggregation; only the second chunk is exposed.  The table is
    laid out chunk-major [A-rows of all cores | B-rows of all cores] and
    int16 gather buckets coincide with half-chunks.
  - aggregation: edges grouped by 128-wide dst windows (GW windows per
    gather group); per-(window,bucket) runs are packed back-to-back at
    exact (union-max) length; 128-slot gather columns may span adjacent
    windows -- each window matmuls every column its run overlaps, with its
    own one-hot S instance (built on DVE from compile-time dst metadata;
    foreign slots get the -1 sentinel and route nowhere).
  - window epilogue: PSUM->SBUF copy, f32 matmul by W, then one ACT
    relu-with-scale (nd*ns folded; biases are zero per the problem spec)
    emits the next layer's bf16 table rows node-major.
"""
import sys
sys.path.insert(0, '/opt/trn_rl_repo')

import numpy as np

import concourse.bass as bass
import concourse.bacc as bacc
import concourse.tile as tile
import concourse.mybir as mybir
from concourse.bass_utils import run_bass_kernel_spmd

f32 = mybir.dt.float32
bf16 = mybir.dt.bfloat16
i16 = mybir.dt.int16

NC = 8
GW = 4                 # dst windows per gather group
N_NODES = 100000
SHARD = N_NODES // NC          # 12500
NW = (SHARD + 127) // 128      # 98
PADSHARD = NW * 128            # 12544
NPAD = NC * PADSHARD           # 100352
NBUK = 4

# AllGather chunks: windows [0, WA) and [WA, NW)
WA = 48
A_ROWS = WA * 128              # 6144 per core
B_ROWS = PADSHARD - A_ROWS     # 6400 per core
# table layout: [all cores' A rows | all cores' B rows]
A_BASE = 0
B_BASE = NC * A_ROWS           # 49152
# int16 gather buckets tile the table; each must be <= 32768 rows
BUK_BASE = [0, NC // 2 * A_ROWS, B_BASE, B_BASE + NC // 2 * B_ROWS]
BUK_ROWS = [NC // 2 * A_ROWS, NC // 2 * A_ROWS,
            NC // 2 * B_ROWS, NC // 2 * B_ROWS]
# AG chunk A is triggered after this group (slack past the producing
# windows so the gpsimd stream never waits on epilogue stores)
AG_A_GROUP = (WA - 1) // GW + 3


def _node_pos(c, w, slot):
    """Global table row of (core, window, slot) in the chunk-major layout."""
    if w < WA:
        return A_BASE + c * A_ROWS + w * 128 + slot
    return B_BASE + c * B_ROWS + (w - WA) * 128 + slot


def _balance_windows(dv):
    """Greedily pack SHARD nodes (with per-bucket degree 4-vectors dv) into
    NW windows of 128 so per-(window,bucket) sums are level.

    Returns (w, slot) arrays of length SHARD.
    """
    order = np.argsort(-dv.sum(axis=1), kind='stable')
    target = dv.sum(axis=0).astype(np.float64) / NW
    loads = np.zeros((NW, NBUK))
    fill = np.zeros(NW, dtype=np.int64)
    w_of = np.zeros(SHARD, dtype=np.int64)
    slot_of = np.zeros(SHARD, dtype=np.int64)
    cap = np.full(NW, 128, dtype=np.int64)
    for i in order:
        d = dv[i]
        # penalty: resulting max overload across buckets
        over = np.maximum(loads + d[None, :] - target[None, :], 0.0).sum(axis=1)
        over[fill >= cap] = np.inf
        w = int(np.argmin(over))
        w_of[i] = w
        slot_of[i] = fill[w]
        fill[w] += 1
        loads[w] += d
    return w_of, slot_of


def _preprocess(src, dst):
    src = np.asarray(src).astype(np.int64)
    dst = np.asarray(dst).astype(np.int64)

    outdeg = np.bincount(src, minlength=N_NODES)
    indeg = np.bincount(dst, minlength=N_NODES)
    ns = (1.0 / np.sqrt(np.maximum(outdeg, 1))).astype(np.float32)
    nd = (1.0 / np.sqrt(np.maximum(indeg, 1))).astype(np.float32)

    # ---- pass 1: provisional src bucket from contiguous layout is not
    # usable (positions depend on the packing), so balance on src-core
    # halves instead: bucket of src = which quarter of the table it lands
    # in, which is determined by (src core, src window < WA).  The packing
    # of the SRC core decides A/B membership; to keep this tractable we
    # balance per core on the 4-vector of (src core pair, A/B) after a
    # provisional packing of all cores.  Two rounds: first pack on src-core
    # pair counts (bucket-invariant under repacking of other cores at the
    # half level is not exact, but counts move little), then compute final
    # buckets from the actual packing.
    c_s = src // SHARD
    r_s = src % SHARD
    ecore = dst // SHARD

    # provisional per-node degree vectors over 4 provisional buckets
    # (src core pairs 01/23/45/67), which repacking never changes.
    prov_b = c_s // 2
    w_of = np.zeros((NC, SHARD), dtype=np.int64)
    slot_of = np.zeros((NC, SHARD), dtype=np.int64)
    for c in range(NC):
        m = ecore == c
        ld = dst[m] - c * SHARD
        dv = np.zeros((SHARD, NBUK), dtype=np.int64)
        np.add.at(dv, (ld, prov_b[m]), 1)
        w_of[c], slot_of[c] = _balance_windows(dv)

    # final positions of every node under the packing
    pos = np.zeros(N_NODES, dtype=np.int64)
    for c in range(NC):
        loc = np.arange(SHARD)
        w = w_of[c]; s = slot_of[c]
        in_a = w < WA
        p = np.where(in_a,
                     A_BASE + c * A_ROWS + w * 128 + s,
                     B_BASE + c * B_ROWS + (w - WA) * 128 + s)
        pos[c * SHARD + loc] = p

    src_pos = pos[src]
    b_s = np.searchsorted(np.asarray(BUK_BASE) + np.asarray(BUK_ROWS),
                          src_pos, side='right')
    reb_s = src_pos - np.asarray(BUK_BASE)[b_s]

    cores = []
    counts = np.zeros((NC, NW, NBUK), dtype=np.int64)
    for c in range(NC):
        m = ecore == c
        ld = dst[m] - c * SHARD
        w = w_of[c][ld]
        slot_d = slot_of[c][ld]
        b = b_s[m]
        reb = reb_s[m]
        order = np.lexsort((reb, b, w))
        w, b, slot_d, reb = w[order], b[order], slot_d[order], reb[order]
        key = w * NBUK + b
        cnt = np.bincount(key, minlength=NW * NBUK).reshape(NW, NBUK)
        counts[c] = cnt
        cores.append((w, b, slot_d, reb, key))

    # exact union-max run lengths
    CX = counts.max(axis=0)

    NG = (NW + GW - 1) // GW
    group_ws = [list(range(g * GW, min((g + 1) * GW, NW))) for g in range(NG)]

    start_x = np.zeros((NW, NBUK), dtype=np.int64)
    calls = []          # [g][b] = (idx_off, n_call, n_cols) or None
    idx_off = 0
    for g, ws in enumerate(group_ws):
        gcalls = []
        for b in range(NBUK):
            acc = 0
            for w in ws:
                start_x[w, b] = acc
                acc += int(CX[w, b])
            if acc == 0:
                gcalls.append(None)
                continue
            n_call = ((acc + 127) // 128) * 128
            gcalls.append((idx_off, n_call, n_call // 128))
            idx_off += n_call
        calls.append(gcalls)
    TOTSLOT = idx_off

    mm_list = []        # [w] = [(b, pc), ...]
    for g, ws in enumerate(group_ws):
        for w in ws:
            lst = []
            for b in range(NBUK):
                if int(CX[w, b]) == 0 or calls[g][b] is None:
                    continue
                s0 = int(start_x[w, b])
                s1 = s0 + int(CX[w, b])
                for pc in range(s0 // 128, (s1 - 1) // 128 + 1):
                    lst.append((b, pc))
            mm_list.append(lst)
    C_w = np.array([len(lst) for lst in mm_list], dtype=np.int64)
    colbase_w = np.zeros(NW, dtype=np.int64)
    colbase_w[1:] = np.cumsum(C_w)[:-1]
    TOTINST = int(C_w.sum())

    inst_of = {}
    for w in range(NW):
        for k, (b, pc) in enumerate(mm_list[w]):
            inst_of[(w, b, pc)] = int(colbase_w[w]) + k

    per_core = []
    for c in range(NC):
        w, b, slot_d, reb, key = cores[c]
        run_start = np.zeros(NW * NBUK, dtype=np.int64)
        run_start[1:] = np.cumsum(np.bincount(key, minlength=NW * NBUK))[:-1]
        p = np.arange(len(key)) - run_start[key]

        g_of_w = w // GW
        call_off = np.array([[calls[g_][b_][0] if calls[g_][b_] else 0
                              for b_ in range(NBUK)]
                             for g_ in range(NG)], dtype=np.int64)
        s = start_x[w, b] + p
        ipos = call_off[g_of_w, b] + s
        pc = s // 128
        sslot = s % 128

        inst = np.array([inst_of[(int(w_), int(b_), int(pc_))]
                         for w_, b_, pc_ in zip(w, b, pc)], dtype=np.int64)

        idx_flat = np.zeros(TOTSLOT, dtype=np.int16)
        idx_flat[ipos] = reb.astype(np.int16)
        dstl = np.full((128, TOTINST), -1.0, dtype=np.float32)
        dstl[sslot, inst] = slot_d.astype(np.float32)

        idx2d = np.tile(idx_flat.reshape(TOTSLOT // 16, 16).T, (8, 1)).copy()

        nsnd_sh = np.zeros(PADSHARD, dtype=np.float32)
        nd_sh = np.zeros(PADSHARD, dtype=np.float32)
        ppos = w_of[c] * 128 + slot_of[c]          # padded local position
        nsnd_sh[ppos] = (ns * nd)[c * SHARD:(c + 1) * SHARD]
        nd_sh[ppos] = nd[c * SHARD:(c + 1) * SHARD]
        nsndcol = nsnd_sh.reshape(NW, 128).T.copy()
        ndcol = nd_sh.reshape(NW, 128).T.copy()

        per_core.append(dict(dstl=dstl, idx=idx2d,
                             nsndcol=nsndcol, ndcol=ndcol))

    st = dict(C_w=C_w, TOTINST=TOTINST, TOTSLOT=TOTSLOT,
              colbase_w=colbase_w, group_ws=group_ws, calls=calls,
              mm_list=mm_list, ns=ns, nd=nd, w_of=w_of, slot_of=slot_of,
              pos=pos)
    return st, per_core


def _build_program(st, f_cls):
    C_w = st['C_w']
    TOTINST, TOTSLOT = st['TOTINST'], st['TOTSLOT']
    colbase_w = st['colbase_w']
    group_ws, calls, mm_list = st['group_ws'], st['calls'], st['mm_list']
    fcp = 64 * ((f_cls + 63) // 64)

    nc = bacc.Bacc(None, target_bir_lowering=False, num_swdge_queues=NBUK)

    hp0_d = nc.dram_tensor("hp0full", [NPAD, 128], bf16, kind="ExternalInput")
    idx_d = nc.dram_tensor("idx16", [128, TOTSLOT // 16], i16, kind="ExternalInput")
    dstl_d = nc.dram_tensor("dstl", [128, TOTINST], bf16, kind="ExternalInput")
    iota_d = nc.dram_tensor("iota", [128, 128], bf16, kind="ExternalInput")
    nsnd_d = nc.dram_tensor("nsndcol", [128, NW], f32, kind="ExternalInput")
    ndcol_d = nc.dram_tensor("ndcol", [128, NW], f32, kind="ExternalInput")
    W0_d = nc.dram_tensor("W0", [128, 128], f32, kind="ExternalInput")
    W1_d = nc.dram_tensor("W1", [128, 128], f32, kind="ExternalInput")
    W2_d = nc.dram_tensor("W2p", [128, fcp], f32, kind="ExternalInput")
    b2_d = nc.dram_tensor("b2rep", [128, fcp], f32, kind="ExternalInput")
    out_d = nc.dram_tensor("out", [PADSHARD, f_cls], f32, kind="ExternalOutput")

    hp1_own = nc.dram_tensor("hp1_own", [PADSHARD, 128], bf16)
    hp2_own = nc.dram_tensor("hp2_own", [PADSHARD, 128], bf16)
    hp1_full = nc.dram_tensor("hp1_full", [NPAD, 128], bf16, addr_space="Shared")
    hp2_full = nc.dram_tensor("hp2_full", [NPAD, 128], bf16, addr_space="Shared")

    rg = [list(range(NC))]

    with tile.TileContext(nc) as tc:
        with (
            tc.tile_pool(name="const", bufs=1) as cpool,
            tc.tile_pool(name="gpool", bufs=5) as gpool,
            tc.tile_pool(name="spool", bufs=6) as spool,
            tc.tile_pool(name="xpool", bufs=6) as xpool,
            tc.tile_pool(name="ipool", bufs=4) as ipool,
            tc.tile_pool(name="psA", bufs=3, space="PSUM") as psA,
            tc.tile_pool(name="psC", bufs=5, space="PSUM") as psC,
        ):
            sW0 = cpool.tile([128, 128], f32); nc.sync.dma_start(sW0[:], W0_d[:])
            sW1 = cpool.tile([128, 128], f32); nc.sync.dma_start(sW1[:], W1_d[:])
            sW2 = cpool.tile([128, fcp], f32); nc.sync.dma_start(sW2[:], W2_d[:])
            sb2 = cpool.tile([128, fcp], f32); nc.sync.dma_start(sb2[:], b2_d[:])
            siota = cpool.tile([128, 128], bf16); nc.sync.dma_start(siota[:], iota_d[:])
            sdstl = cpool.tile([128, TOTINST], bf16); nc.sync.dma_start(sdstl[:], dstl_d[:])
            snsnd = cpool.tile([128, NW], f32); nc.sync.dma_start(snsnd[:], nsnd_d[:])
            sndcol = cpool.tile([128, NW], f32); nc.sync.dma_start(sndcol[:], ndcol_d[:])

            def agg_layer(hp_full, layer):
                sW = (sW0, sW1, sW2)[layer]
                fo = 128 if layer < 2 else fcp
                hp_next_own = (hp1_own, hp2_own, None)[layer]
                hp_next_full = (hp1_full, hp2_full, None)[layer]
                for g, ws in enumerate(group_ws):
                    gcalls = calls[g]
                    idxcols = sum(c[1] // 16 for c in gcalls if c is not None)
                    idxs = ipool.tile([128, max(1, idxcols)], i16, tag="idx")
                    g0 = next(c for c in gcalls if c is not None)[0]
                    nc.sync.dma_start(
                        idxs[:, 0:idxcols],
                        idx_d[:, g0 // 16:(g0 + idxcols * 16) // 16])
                    Gt = []
                    loff = 0
                    for b in range(NBUK):
                        if gcalls[b] is None:
                            Gt.append(None)
                            continue
                        off, n_call, n_cols = gcalls[b]
                        G = gpool.tile([128, n_cols, 128], bf16, tag=f"G{b}")
                        nc.gpsimd.dma_gather(
                            out_ap=G[:, :, :],
                            in_ap=hp_full[BUK_BASE[b]:BUK_BASE[b] + BUK_ROWS[b], :],
                            idxs_ap=idxs[:16, loff:loff + n_call // 16],
                            num_idxs=n_call,
                            num_idxs_reg=n_call,
                            elem_size=128,
                            single_packet=False,
                            queue_num=b,
                        )
                        Gt.append(G)
                        loff += n_call // 16
                    for w in ws:
                        cw = int(C_w[w])
                        cb = int(colbase_w[w])
                        S = spool.tile([128, cw * 128], bf16, tag="S")
                        in0 = sdstl[:, cb:cb + cw].unsqueeze(2).broadcast_to([128, cw, 128])
                        in1 = siota[:, :].unsqueeze(1).broadcast_to([128, cw, 128])
                        nc.vector.tensor_tensor(
                            S[:, :].rearrange("p (c x) -> p c x", x=128),
                            in0, in1, mybir.AluOpType.is_equal)
                        ps = psC.tile([128, 128], f32, tag="psC")
                        for k, (b, pc) in enumerate(mm_list[w]):
                            nc.tensor.matmul(
                                ps[:], Gt[b][:, pc, :],
                                S[:, k * 128:(k + 1) * 128],
                                start=(k == 0), stop=(k == cw - 1))
                        aggT = xpool.tile([128, 128], f32, tag="aggT")
                        nc.scalar.activation(aggT[:], ps[:],
                                             mybir.ActivationFunctionType.Copy)
                        ps2 = psA.tile([128, fo], f32, tag="psA")
                        nc.tensor.matmul(ps2[:], aggT[:], sW[:], start=True, stop=True)
                        if layer < 2:
                            hp = xpool.tile([128, 128], bf16, tag="hp")
                            nc.scalar.activation(hp[:], ps2[:],
                                                 mybir.ActivationFunctionType.Relu,
                                                 scale=snsnd[:, w:w + 1])
                            nc.sync.dma_start(hp_next_own[w * 128:(w + 1) * 128, :], hp[:])
                        else:
                            t = xpool.tile([128, fcp], f32, tag="t2")
                            nc.vector.tensor_scalar_mul(t[:], ps2[:], sndcol[:, w:w + 1])
                            o = xpool.tile([128, fcp], f32, tag="o2")
                            nc.vector.tensor_tensor(o[:], t[:], sb2[:],
                                                    mybir.AluOpType.add)
                            nc.sync.dma_start(out_d[w * 128:(w + 1) * 128, :],
                                              o[:, 0:f_cls])
                    if layer < 2 and g == AG_A_GROUP:
                        nc.gpsimd.collective_compute(
                            "AllGather", mybir.AluOpType.bypass, rg,
                            ins=[hp_next_own[0:A_ROWS, :]],
                            outs=[hp_next_full[A_BASE:A_BASE + NC * A_ROWS, :]])
                if layer < 2:
                    nc.gpsimd.collective_compute(
                        "AllGather", mybir.AluOpType.bypass, rg,
                        ins=[hp_next_own[A_ROWS:PADSHARD, :]],
                        outs=[hp_next_full[B_BASE:B_BASE + NC * B_ROWS, :]])

            agg_layer(hp0_d, 0)
            agg_layer(hp1_full, 1)
            agg_layer(hp2_full, 2)

    nc.compile()
    return nc


_cache = {}


def kernel(feat, src, dst, W0, b0, W1, b1, W2, b2):
    import ml_dtypes
    feat = np.ascontiguousarray(feat, dtype=np.float32)
    f_cls = np.asarray(W2).shape[1]
    fcp = 64 * ((f_cls + 63) // 64)

    key = (hash(np.asarray(src).tobytes()), hash(np.asarray(dst).tobytes()))
    if key in _cache:
        st, per_core, nc_prog = _cache[key]
    else:
        st, per_core = _preprocess(src, dst)
        nc_prog = _build_program(st, f_cls)
        _cache[key] = (st, per_core, nc_prog)

    ns, pos = st['ns'], st['pos']
    # layer-0 table: bf16(feat * ns) scattered to table positions, replicated
    hp0_full = np.zeros((NPAD, 128), dtype=ml_dtypes.bfloat16)
    hp0_full[pos] = (feat * ns[:, None]).astype(ml_dtypes.bfloat16)

    iota = np.tile(np.arange(128, dtype=np.float32), (128, 1))
    W2p = np.zeros((128, fcp), dtype=np.float32)
    W2p[:, :f_cls] = np.asarray(W2, dtype=np.float32)
    b2rep = np.zeros((128, fcp), dtype=np.float32)
    b2rep[:, :f_cls] = np.asarray(b2, dtype=np.float32)[None, :]
    bfv = lambda a: np.ascontiguousarray(a).astype(ml_dtypes.bfloat16)

    in_maps = []
    for c in range(NC):
        pc = per_core[c]
        in_maps.append({
            "hp0full": hp0_full,
            "idx16": pc['idx'],
            "dstl": bfv(pc['dstl']),
            "iota": bfv(iota),
            "nsndcol": pc['nsndcol'],
            "ndcol": pc['ndcol'],
            "W0": np.asarray(W0, dtype=np.float32),
            "W1": np.asarray(W1, dtype=np.float32),
            "W2p": W2p,
            "b2rep": b2rep,
        })

    import os
    trace = os.environ.get("GCN_TRACE") == "1"
    res = run_bass_kernel_spmd(nc_prog, in_maps, core_ids=list(range(NC)),
                               trace=trace)
    global last_results
    last_results = res
    # un-permute: node (c, i) lives at padded row w*128+slot of core c's out
    w_of, slot_of = st['w_of'], st['slot_of']
    out = np.empty((N_NODES, f_cls), dtype=np.float32)
    for c in range(NC):
        ppos = w_of[c] * 128 + slot_of[c]
        out[c * SHARD:(c + 1) * SHARD] = res.results[c]["out"][ppos]
    return np.ascontiguousarray(out, dtype=np.float32)


last_results = None
